# revision 26
# baseline (speedup 1.0000x reference)
"""Trainium2 Bass kernel for a single-step Elman RNN cell + linear + softmax.

Reference computation (B=256, I=H=O=4096, fp32):
    hn     = tanh(x @ w_ih.T + b_ih + h0[0] @ w_hh.T + b_hh)      # [B, H]
    logits = hn @ w_lin.T + b_lin                                  # [B, O]
    probs  = softmax(logits, axis=-1)
    return probs[None], hn[None]

Sharding (8 cores, tensor-parallel): core c owns rows hs = [512c, 512c+512)
of H (and the same slice of O).
  Phase 1: each core computes hnT_c = tanh(W_ih[hs] @ x.T + W_hh[hs] @ h.T + b)
           as [512, 256] (H on partitions, batch on free dim), in G column
           groups; each group's [256, 256] result is AllGathered while the
           next group (and then phase 2) computes, hiding collective latency.
  Phase 2: each core computes its O-slice of logits: [256, 512] =
           (hnT k-tiles).T @ w_lin[os].T, batch on partitions.
  Softmax: exp on-chip; per-core partial row sums are AllGathered (tiny) and
           summed so every core normalizes its O-slice with the global denom.

All matmul operands are pre-transposed on the host so the contraction dim (I
resp. H) lands on SBUF partitions and every DMA is contiguous.

hnT SBUF layout: the gathered hnT k-tiles are stored in (group, rank, kk)
order, index j = g*(KT/G) + r*(KT/(G*NCORES... )); phase 2 maps global k-tile
k = 4r + 2g + kk  ->  j = g*16 + r*2 + kk (for G=2) so each group's AllGather
lands contiguously while matmuls read the right tile.
"""

import os

import numpy as np

import concourse.bass as bass
import concourse.mybir as mybir
import concourse.tile as tile
from concourse import bacc
from concourse.bass import ts
from concourse.bass_utils import run_bass_kernel_spmd
from concourse.tile_rust import add_dep_helper

NCORES = 8
B = 256
I = H = O = 4096
SH = H // NCORES  # 512: per-core shard of H / O
P = 128
KT = I // P  # 32 k-tiles
MS = SH // P  # 4 m-tiles (H-shard)
BT = B // P  # 2 batch tiles
# Phase-1 gather groups (m-tiles per AllGather). Two groups measured best:
# collective ops are latency-bound (~14us + ~6us/MB) and serialize on the
# cc stream (a doorbell can't ring until the previous op completes), so
# more groups cost more than the overlap they buy; one group delays the
# first doorbell until all of phase 1 is done.
GROUPS = [[0, 1], [2, 3]]
G = len(GROUPS)

F32 = mybir.dt.float32
BF16 = mybir.dt.bfloat16
FP16 = mybir.dt.float16

# Matmul precision mode: "fp32" (exact, 4 cyc/row), "fp16" (1 cyc/row,
# ~5e-4 rel err), "bf16" (1 cyc/row, ~3e-3 rel err).
MODE = os.environ.get("RNN_MODE", "fp16")

# k-tile groups per slab DMA: uniform medium slabs keep several DMAs in
# flight from the first issue to the end of each class (a lone in-flight
# DMA only sustains ~50 GB/s, and tiny leading slabs waste ramp time).
P1_SLABS = [4, 4, 4, 4, 4, 4, 2, 2, 2, 2]
PW_SLABS = [8, 8, 8, 8]  # later weight classes: 4 slabs per stream
PL_SLABS = [4, 4, 4, 4, 4, 4, 4, 4]  # wlin

_cache: dict = {}


def _mm_dt(mode):
    return {"fp32": F32, "bf16": BF16, "fp16": FP16}[mode]


def _emit(nc, tc, mode):
    mdt = _mm_dt(mode)

    # ---- DRAM I/O ----
    xT = nc.dram_tensor("xT", [I, B], mdt, kind="ExternalInput")
    hT = nc.dram_tensor("hT", [H, B], mdt, kind="ExternalInput")
    wih = nc.dram_tensor("wih", [I, SH], mdt, kind="ExternalInput")
    whh = nc.dram_tensor("whh", [H, SH], mdt, kind="ExternalInput")
    wlin = nc.dram_tensor("wlin", [H, SH], mdt, kind="ExternalInput")
    blin = nc.dram_tensor("blin", [1, SH], mdt, kind="ExternalInput")
    b1 = nc.dram_tensor("b1", [P, MS], F32, kind="ExternalInput")  # (b_ih+b_hh)[hs]

    odt = F32 if mode == "fp32" else FP16
    probs_out = nc.dram_tensor("probs_s", [B, SH], odt, kind="ExternalOutput")
    hn_out = nc.dram_tensor("hn_s", [SH, B], F32, kind="ExternalOutput")

    rg = [list(range(NCORES))]

    with (
        tc.tile_pool(name="const", bufs=1) as const_pool,
        tc.tile_pool(name="acts", bufs=1) as acts_pool,
        tc.tile_pool(name="ps1", bufs=1, space="PSUM") as ps1_pool,
        tc.tile_pool(name="ps2", bufs=1, space="PSUM") as ps2_pool,
        tc.tile_pool(name="dram", bufs=1, space="DRAM") as dram_pool,
    ):
        # ---- constants ----
        b1_sb = const_pool.tile([P, MS], F32)
        nc.sync.dma_start(b1_sb[:], b1.ap())
        blin_sb = const_pool.tile([1, SH], mdt)
        nc.sync.dma_start(blin_sb[:], blin.ap())
        ones_sb = const_pool.tile([1, B], mdt)
        nc.vector.memset(ones_sb[:], 1.0)

        # ---- resident activations ----
        xT_sb = acts_pool.tile([P, KT, B], mdt)
        hT_sb = acts_pool.tile([P, KT, B], mdt)
        wih_sb = acts_pool.tile([P, KT, SH], mdt)  # resident phase-1 weights
        whh_sb = acts_pool.tile([P, KT, SH], mdt)
        hnT_sb = acts_pool.tile([P, KT, B], mdt)  # gathered full hnT (j-order)
        hn32_sb = acts_pool.tile([P, MS, B], F32)  # own shard, fp32 (output)
        if mode == "fp32":
            hnmm_sb = hn32_sb
        else:
            hnmm_sb = acts_pool.tile([P, MS, B], mdt)

        # collective bounce buffers (one pair per phase-1 group)
        cc1_in = []
        cc1_out = []
        for g, mts in enumerate(GROUPS):
            gw = len(mts) * P
            t_in = dram_pool.tile([gw, B], mdt, name=f"cc1_in_{g}")
            t_out = dram_pool.tile(
                [gw * NCORES, B], mdt, addr_space="Shared", name=f"cc1_out_{g}"
            )
            cc1_in.append(t_in)
            cc1_out.append(t_out)
        # softmax denominator exchange: partition-major [P, BT] layout so the
        # 1KB store/load is 128 x 8B descriptors, AllReduce(add) so no
        # on-chip 8-way reduce is needed afterwards.
        cc2_in = dram_pool.tile([P, BT], F32)
        cc2_out = dram_pool.tile([P, BT], F32)

        # ---- phase 1, grouped; group g covers shard columns [g*GW,(g+1)*GW) ----
        ps1 = [
            ps1_pool.tile([P, B], F32, tag=f"ps1_{m}", name=f"ps1_{m}")
            for m in range(MS)
        ]

        # Phase-1 operand streaming. HBM aggregate is ~225 GB/s shared
        # pro-rata by whatever DMAs are in flight, so class sequencing is
        # everything: class g streams the wih/whh columns of gather-group g
        # (class 0 also carries x and h); the last class streams wlin. Each
        # class's first slabs carry dependency edges on the previous class's
        # last slabs so later classes never steal earlier-class bandwidth.
        # x+wih go on sync, h+whh on scalar for issue-rate headroom; wlin on
        # sync (scalar must stay clear for tanh by the time class 0 lands).
        col0 = [sum(len(m) for m in GROUPS[:g]) * P for g in range(G + 1)]
        prev_last: list = []
        pos = 0
        for nk in P1_SLABS:
            ksl = slice(pos * P, (pos + nk) * P)
            gsl = slice(col0[0], col0[1])
            i1 = nc.sync.dma_start(
                xT_sb[:, pos : pos + nk, :],
                xT.ap()[ksl, :].rearrange("(kk p) b -> p kk b", p=P),
            )
            i2 = nc.scalar.dma_start(
                hT_sb[:, pos : pos + nk, :],
                hT.ap()[ksl, :].rearrange("(kk p) b -> p kk b", p=P),
            )
            i3 = nc.sync.dma_start(
                wih_sb[:, pos : pos + nk, gsl],
                wih.ap()[ksl, gsl].rearrange("(kk p) s -> p kk s", p=P),
            )
            i4 = nc.scalar.dma_start(
                whh_sb[:, pos : pos + nk, gsl],
                whh.ap()[ksl, gsl].rearrange("(kk p) s -> p kk s", p=P),
            )
            pos += nk
            if pos == KT:
                prev_last = [i1, i2, i3, i4]
        wlin_sb = acts_pool.tile([P, KT, SH], mdt)

        def stream_wclass(g, deps):
            # stream wih/whh columns of group g in slabs; first slabs
            # depend on `deps` (previous class's last slabs + the doorbell
            # store, so the tiny store DMA isn't starved of bandwidth)
            gsl = slice(col0[g], col0[g + 1])
            cur_last = []
            pos = 0
            for nk in PW_SLABS:
                ksl = slice(pos * P, (pos + nk) * P)
                i1 = nc.sync.dma_start(
                    wih_sb[:, pos : pos + nk, gsl],
                    wih.ap()[ksl, gsl].rearrange("(kk p) s -> p kk s", p=P),
                )
                i2 = nc.sync.dma_start(
                    whh_sb[:, pos : pos + nk, gsl],
                    whh.ap()[ksl, gsl].rearrange("(kk p) s -> p kk s", p=P),
                )
                if pos == 0:
                    for a in deps:
                        add_dep_helper(i1.ins, a.ins, reason="class order")
                        add_dep_helper(i2.ins, a.ins, reason="class order")
                pos += nk
                if pos == KT:
                    cur_last = [i1, i2]
            return cur_last

        def stream_wlin(deps):
            pos = 0
            for nk in PL_SLABS:
                ksl = slice(pos * P, (pos + nk) * P)
                i1 = nc.sync.dma_start(
                    wlin_sb[:, pos : pos + nk, :],
                    wlin.ap()[ksl, :].rearrange("(kk p) s -> p kk s", p=P),
                )
                if pos == 0:
                    for b in deps:
                        add_dep_helper(i1.ins, b.ins, reason="class order")
                pos += nk

        jbase = 0
        for g, mts in enumerate(GROUPS):
            for k in range(KT):
                for m in mts:
                    nc.tensor.matmul(
                        ps1[m][:],
                        lhsT=wih_sb[:, k, ts(m, P)],
                        rhs=xT_sb[:, k, :],
                        start=(k == 0),
                        stop=False,
                    )
                    nc.tensor.matmul(
                        ps1[m][:],
                        lhsT=whh_sb[:, k, ts(m, P)],
                        rhs=hT_sb[:, k, :],
                        start=False,
                        stop=(k == KT - 1),
                    )

            for m in mts:
                nc.scalar.activation(
                    hn32_sb[:, m, :],
                    ps1[m][:],
                    mybir.ActivationFunctionType.Tanh,
                    bias=b1_sb[:, m : m + 1],
                )
                if mode != "fp32":
                    nc.scalar.activation(
                        hnmm_sb[:, m, :],
                        ps1[m][:],
                        mybir.ActivationFunctionType.Tanh,
                        bias=b1_sb[:, m : m + 1],
                    )

            # group shard -> DRAM -> AllGather. Store split across gpsimd
            # and scalar so the two halves' completions retire in parallel
            # (the doorbell waits on both via the collective's input deps).
            cc1_pview = cc1_in[g].rearrange("(m p) b -> p m b", p=P)
            nm = len(mts)
            mh = nm // 2 if nm > 1 else 1
            st = nc.gpsimd.dma_start(
                cc1_pview[:, :mh, :], hnmm_sb[:, mts[0] : mts[0] + mh, :]
            )
            if nm > mh:
                nc.scalar.dma_start(
                    cc1_pview[:, mh:, :], hnmm_sb[:, mts[0] + mh : mts[-1] + 1, :]
                )
            nc.gpsimd.collective_compute(
                "AllGather",
                mybir.AluOpType.bypass,
                replica_groups=rg,
                ins=[cc1_in[g][:]],
                outs=[cc1_out[g][:]],
            )
            # next streaming class (the store->doorbell latency is a fixed
            # ~8us completion cost, so don't serialize classes behind it)
            if g + 1 < G:
                prev_last = stream_wclass(g + 1, prev_last)
            else:
                stream_wlin(prev_last)

        # own hn shard is final now; store it early (overlaps phase 2).
        nc.scalar.dma_start(hn_out.ap().rearrange("(m p) b -> p m b", p=P), hn32_sb[:])

        # gathered j-slots -> SBUF, emitted after both doorbells so the g1
        # store/doorbell is never queue-blocked behind a copy head-wait.
        # Each group's copy is chunked across scalar and gpsimd so the first
        # phase-2 matmuls can start ~1us after the gather lands instead of
        # waiting for the full 1MB copy.
        jbase = 0
        for g, mts in enumerate(GROUPS):
            nj = NCORES * len(mts)  # j-slots this group
            cc1_view = cc1_out[g].rearrange("(rk p) b -> p rk b", p=P)
            # first chunks tiny so phase-2 matmuls resume ~0.5us after the
            # gather lands; j-slots are consumed in order by phase 2
            sizes = [1, 1, 3, 3, 4, 4] if nj == 16 else [nj // 2, nj - nj // 2]
            lo = 0
            for ci, cw in enumerate(sizes):
                eng = nc.gpsimd if ci % 2 == 0 else nc.scalar
                eng.dma_start(
                    hnT_sb[:, jbase + lo : jbase + lo + cw, :],
                    cc1_view[:, lo : lo + cw, :],
                )
                lo += cw
            jbase += nj

        # ---- phase 2: logits_c = hnT.T @ wlin (+ blin via ones-row) ----
        # hnT_sb is in j-order (group-major), and the host pre-permutes wlin's
        # rows into the same j-order, so iterating j consumes group 0's tiles
        # first (phase 2 starts as soon as AllGather 0 lands). wlin is fully
        # SBUF-resident: its 4 big DMAs queue on sync behind the phase-1
        # slabs and stream during the AllGather window when HBM is otherwise
        # idle. The bias (ones-row) matmul OPENS each accumulation group so
        # the group closes on the last j-matmul and exp can start immediately.
        ps2 = [
            ps2_pool.tile([P, SH], F32, tag=f"ps2_{mb}", name=f"ps2_{mb}")
            for mb in range(BT)
        ]
        for mb in range(BT):
            nc.tensor.matmul(
                ps2[mb][:],
                lhsT=ones_sb[:, ts(mb, P)],
                rhs=blin_sb[:],
                start=True,
                stop=False,
            )
        for j in range(KT):
            for mb in range(BT):
                nc.tensor.matmul(
                    ps2[mb][:],
                    lhsT=hnT_sb[:, j, ts(mb, P)],
                    rhs=wlin_sb[:, j, :],
                    start=False,
                    stop=(j == KT - 1),
                )

        # ---- softmax over full O (partial sums summed via AllReduce) ----
        odt = F32 if mode == "fp32" else FP16  # probs output dtype
        probs_sb = acts_pool.tile([P, BT, SH], F32)
        probs_o_sb = acts_pool.tile([P, BT, SH], odt)
        part_sb = acts_pool.tile([P, BT], F32)
        den_sb = acts_pool.tile([P, BT], F32)
        rden_sb = acts_pool.tile([P, BT], F32)

        for mb in range(BT):
            nc.scalar.activation(
                probs_sb[:, mb, :], ps2[mb][:], mybir.ActivationFunctionType.Exp
            )
            nc.vector.reduce_sum(
                part_sb[:, mb : mb + 1], probs_sb[:, mb, :], axis=mybir.AxisListType.X
            )
        nc.gpsimd.dma_start(cc2_in[:], part_sb[:])
        nc.gpsimd.collective_compute(
            "AllReduce",
            mybir.AluOpType.add,
            replica_groups=rg,
            ins=[cc2_in[:]],
            outs=[cc2_out[:]],
        )
        nc.gpsimd.dma_start(den_sb[:], cc2_out[:])
        nc.vector.reciprocal(rden_sb[:], den_sb[:])
        probs_view = probs_out.ap().rearrange("(m p) o -> p m o", p=P)
        for mb in range(BT):
            nc.vector.tensor_scalar_mul(
                probs_o_sb[:, mb, :], probs_sb[:, mb, :], rden_sb[:, mb : mb + 1]
            )
            nc.sync.dma_start(probs_view[:, mb, :], probs_o_sb[:, mb, :])


def _build(mode):
    if mode in _cache:
        return _cache[mode]
    nc = bacc.Bacc(
        "TRN2",
        target_bir_lowering=False,
        debug=False,
        num_devices=NCORES,
    )
    with tile.TileContext(nc) as tc:
        _emit(nc, tc, mode)
    nc.compile()
    _cache[mode] = nc
    return nc


def _np_dt(mode):
    if mode == "bf16":
        import ml_dtypes

        return ml_dtypes.bfloat16
    if mode == "fp16":
        return np.float16
    return np.float32


def _prep_in_maps(x, h0, w_ih, b_ih, w_hh, b_hh, w_lin, b_lin, mode):
    dt = _np_dt(mode)
    x = np.asarray(x, np.float32)
    h = np.asarray(h0, np.float32).reshape(B, H)
    w_ih = np.asarray(w_ih, np.float32)
    w_hh = np.asarray(w_hh, np.float32)
    w_lin = np.asarray(w_lin, np.float32)
    b1_full = np.asarray(b_ih, np.float32) + np.asarray(b_hh, np.float32)
    b_lin = np.asarray(b_lin, np.float32)

    xT = np.ascontiguousarray(x.T).astype(dt, copy=False)
    hT = np.ascontiguousarray(h.T).astype(dt, copy=False)

    in_maps = []
    for c in range(NCORES):
        hs = slice(c * SH, (c + 1) * SH)
        # wlin rows permuted to match hnT_sb's j-order: group-major, then
        # rank, then the group's m-tiles; global k-tile k = MS*r + m.
        wlt = np.ascontiguousarray(w_lin[hs].T).astype(dt, copy=False)
        blocks = []
        for mts in GROUPS:
            for r in range(NCORES):
                for m in mts:
                    k = MS * r + m
                    blocks.append(wlt[k * P : (k + 1) * P])
        wlt_j = np.ascontiguousarray(np.concatenate(blocks, axis=0))
        in_maps.append(
            {
                "xT": xT,
                "hT": hT,
                "wih": np.ascontiguousarray(w_ih[hs].T).astype(dt, copy=False),
                "whh": np.ascontiguousarray(w_hh[hs].T).astype(dt, copy=False),
                "wlin": wlt_j,
                "blin": np.ascontiguousarray(b_lin[hs][None, :]).astype(dt, copy=False),
                "b1": np.ascontiguousarray(b1_full[hs].reshape(MS, P).T),
            }
        )
    return in_maps


def _gather(results):
    probs = np.concatenate(
        [np.asarray(results[c]["probs_s"], np.float32) for c in range(NCORES)], axis=1
    )
    hnT = np.concatenate([results[c]["hn_s"] for c in range(NCORES)], axis=0)
    hn = np.ascontiguousarray(hnT.T)
    return probs[None, :, :], hn[None, :, :]


def run(inputs, mode=None, **spmd_kwargs):
    mode = mode or MODE
    nc = _build(mode)
    in_maps = _prep_in_maps(**inputs, mode=mode)
    res = run_bass_kernel_spmd(nc, in_maps, core_ids=list(range(NCORES)), **spmd_kwargs)
    return _gather(res.results), res


def kernel(x, h0, w_ih, b_ih, w_hh, b_hh, w_lin, b_lin):
    out, _ = run(
        dict(
            x=x, h0=h0, w_ih=w_ih, b_ih=b_ih, w_hh=w_hh, b_hh=b_hh,
            w_lin=w_lin, b_lin=b_lin,
        )
    )
    return out



# revision 30
# speedup vs baseline: 1.0385x; 1.0385x over previous
"""Trainium2 Bass kernel for a single-step Elman RNN cell + linear + softmax.

Reference computation (B=256, I=H=O=4096, fp32):
    hn     = tanh(x @ w_ih.T + b_ih + h0[0] @ w_hh.T + b_hh)      # [B, H]
    logits = hn @ w_lin.T + b_lin                                  # [B, O]
    probs  = softmax(logits, axis=-1)
    return probs[None], hn[None]

Sharding (8 cores, tensor-parallel): core c owns rows hs = [512c, 512c+512)
of H (and the same slice of O).
  Phase 1: each core computes hnT_c = tanh(W_ih[hs] @ x.T + W_hh[hs] @ h.T + b)
           as [512, 256] (H on partitions, batch on free dim), in G column
           groups; each group's [256, 256] result is AllGathered while the
           next group (and then phase 2) computes, hiding collective latency.
  Phase 2: each core computes its O-slice of logits: [256, 512] =
           (hnT k-tiles).T @ w_lin[os].T, batch on partitions.
  Softmax: exp on-chip; per-core partial row sums are AllGathered (tiny) and
           summed so every core normalizes its O-slice with the global denom.

All matmul operands are pre-transposed on the host so the contraction dim (I
resp. H) lands on SBUF partitions and every DMA is contiguous.

hnT SBUF layout: the gathered hnT k-tiles are stored in (group, rank, kk)
order, index j = g*(KT/G) + r*(KT/(G*NCORES... )); phase 2 maps global k-tile
k = 4r + 2g + kk  ->  j = g*16 + r*2 + kk (for G=2) so each group's AllGather
lands contiguously while matmuls read the right tile.
"""

import os

import numpy as np

import concourse.bass as bass
import concourse.mybir as mybir
import concourse.tile as tile
from concourse import bacc
from concourse.bass import ts
from concourse.bass_utils import run_bass_kernel_spmd
from concourse.tile_rust import add_dep_helper

NCORES = 8
B = 256
I = H = O = 4096
SH = H // NCORES  # 512: per-core shard of H / O
P = 128
KT = I // P  # 32 k-tiles
MS = SH // P  # 4 m-tiles (H-shard)
BT = B // P  # 2 batch tiles
# Phase-1 gather groups (m-tiles per AllGather). Two groups measured best:
# collective ops are latency-bound (~14us + ~6us/MB) and serialize on the
# cc stream (a doorbell can't ring until the previous op completes), so
# more groups cost more than the overlap they buy; one group delays the
# first doorbell until all of phase 1 is done.
GROUPS = [[0, 1], [2, 3]]
G = len(GROUPS)

F32 = mybir.dt.float32
BF16 = mybir.dt.bfloat16
FP16 = mybir.dt.float16

# Matmul precision mode: "fp32" (exact, 4 cyc/row), "fp16" (1 cyc/row,
# ~5e-4 rel err), "bf16" (1 cyc/row, ~3e-3 rel err).
MODE = os.environ.get("RNN_MODE", "fp16")

# k-tile groups per slab DMA: uniform medium slabs keep several DMAs in
# flight from the first issue to the end of each class (a lone in-flight
# DMA only sustains ~50 GB/s, and tiny leading slabs waste ramp time).
P1_SLABS = [4, 4, 4, 4, 4, 4, 2, 2, 2, 2]
PW_SLABS = [8, 8, 8, 8]  # later weight classes: 4 slabs per stream
PL_SLABS = [4, 4, 4, 4, 4, 4, 4, 4]  # wlin

_cache: dict = {}


def _mm_dt(mode):
    return {"fp32": F32, "bf16": BF16, "fp16": FP16}[mode]


def _emit(nc, tc, mode):
    mdt = _mm_dt(mode)

    # ---- DRAM I/O ----
    xT = nc.dram_tensor("xT", [I, B], mdt, kind="ExternalInput")
    hT = nc.dram_tensor("hT", [H, B], mdt, kind="ExternalInput")
    wih = nc.dram_tensor("wih", [I, SH], mdt, kind="ExternalInput")
    whh = nc.dram_tensor("whh", [H, SH], mdt, kind="ExternalInput")
    wlin = nc.dram_tensor("wlin", [H, SH], mdt, kind="ExternalInput")
    blin = nc.dram_tensor("blin", [1, SH], mdt, kind="ExternalInput")
    b1 = nc.dram_tensor("b1", [P, MS], F32, kind="ExternalInput")  # (b_ih+b_hh)[hs]

    odt = F32 if mode == "fp32" else FP16
    probs_out = nc.dram_tensor("probs_s", [B, SH], odt, kind="ExternalOutput")
    hn_out = nc.dram_tensor("hn_s", [SH, B], F32, kind="ExternalOutput")

    rg = [list(range(NCORES))]

    with (
        tc.tile_pool(name="const", bufs=1) as const_pool,
        tc.tile_pool(name="acts", bufs=1) as acts_pool,
        tc.tile_pool(name="ps1", bufs=1, space="PSUM") as ps1_pool,
        tc.tile_pool(name="ps2", bufs=1, space="PSUM") as ps2_pool,
        tc.tile_pool(name="dram", bufs=1, space="DRAM") as dram_pool,
    ):
        # ---- constants ----
        b1_sb = const_pool.tile([P, MS], F32)
        nc.sync.dma_start(b1_sb[:], b1.ap())
        blin_sb = const_pool.tile([1, SH], mdt)
        nc.sync.dma_start(blin_sb[:], blin.ap())
        ones_sb = const_pool.tile([1, B], mdt)
        nc.vector.memset(ones_sb[:], 1.0)

        # ---- resident activations ----
        xT_sb = acts_pool.tile([P, KT, B], mdt)
        hT_sb = acts_pool.tile([P, KT, B], mdt)
        wih_sb = acts_pool.tile([P, KT, SH], mdt)  # resident phase-1 weights
        whh_sb = acts_pool.tile([P, KT, SH], mdt)
        hnT_sb = acts_pool.tile([P, KT, B], mdt)  # gathered full hnT (j-order)
        hn32_sb = acts_pool.tile([P, MS, B], F32)  # own shard, fp32 (output)
        if mode == "fp32":
            hnmm_sb = hn32_sb
        else:
            hnmm_sb = acts_pool.tile([P, MS, B], mdt)

        # collective bounce buffers (one pair per phase-1 group)
        cc1_in = []
        cc1_out = []
        for g, mts in enumerate(GROUPS):
            gw = len(mts) * P
            t_in = dram_pool.tile([gw, B], mdt, name=f"cc1_in_{g}")
            t_out = dram_pool.tile(
                [gw * NCORES, B], mdt, addr_space="Shared", name=f"cc1_out_{g}"
            )
            cc1_in.append(t_in)
            cc1_out.append(t_out)
        # softmax denominator exchange: partition-major [P, BT] layout so the
        # 1KB store/load is 128 x 8B descriptors, AllReduce(add) so no
        # on-chip 8-way reduce is needed afterwards.
        cc2_in = dram_pool.tile([P, BT], F32)
        cc2_out = dram_pool.tile([P, BT], F32)

        # ---- phase 1, grouped; group g covers shard columns [g*GW,(g+1)*GW) ----
        ps1 = [
            ps1_pool.tile([P, B], F32, tag=f"ps1_{m}", name=f"ps1_{m}")
            for m in range(MS)
        ]

        # Phase-1 operand streaming. HBM aggregate is ~225 GB/s shared
        # pro-rata by whatever DMAs are in flight, so class sequencing is
        # everything: class g streams the wih/whh columns of gather-group g
        # (class 0 also carries x and h); the last class streams wlin. Each
        # class's first slabs carry dependency edges on the previous class's
        # last slabs so later classes never steal earlier-class bandwidth.
        # x+wih go on sync, h+whh on scalar for issue-rate headroom; wlin on
        # sync (scalar must stay clear for tanh by the time class 0 lands).
        col0 = [sum(len(m) for m in GROUPS[:g]) * P for g in range(G + 1)]
        prev_last: list = []
        pos = 0
        for nk in P1_SLABS:
            ksl = slice(pos * P, (pos + nk) * P)
            gsl = slice(col0[0], col0[1])
            i1 = nc.sync.dma_start(
                xT_sb[:, pos : pos + nk, :],
                xT.ap()[ksl, :].rearrange("(kk p) b -> p kk b", p=P),
            )
            i2 = nc.scalar.dma_start(
                hT_sb[:, pos : pos + nk, :],
                hT.ap()[ksl, :].rearrange("(kk p) b -> p kk b", p=P),
            )
            i3 = nc.sync.dma_start(
                wih_sb[:, pos : pos + nk, gsl],
                wih.ap()[ksl, gsl].rearrange("(kk p) s -> p kk s", p=P),
            )
            i4 = nc.scalar.dma_start(
                whh_sb[:, pos : pos + nk, gsl],
                whh.ap()[ksl, gsl].rearrange("(kk p) s -> p kk s", p=P),
            )
            pos += nk
            if pos == KT:
                prev_last = [i1, i2, i3, i4]
        wlin_sb = acts_pool.tile([P, KT, SH], mdt)

        def stream_wclass(g, deps):
            # stream wih/whh columns of group g in slabs; first slabs
            # depend on `deps` (previous class's last slabs + the doorbell
            # store, so the tiny store DMA isn't starved of bandwidth)
            gsl = slice(col0[g], col0[g + 1])
            cur_last = []
            pos = 0
            for nk in PW_SLABS:
                ksl = slice(pos * P, (pos + nk) * P)
                i1 = nc.sync.dma_start(
                    wih_sb[:, pos : pos + nk, gsl],
                    wih.ap()[ksl, gsl].rearrange("(kk p) s -> p kk s", p=P),
                )
                i2 = nc.sync.dma_start(
                    whh_sb[:, pos : pos + nk, gsl],
                    whh.ap()[ksl, gsl].rearrange("(kk p) s -> p kk s", p=P),
                )
                if pos == 0:
                    for a in deps:
                        add_dep_helper(i1.ins, a.ins, reason="class order")
                        add_dep_helper(i2.ins, a.ins, reason="class order")
                pos += nk
                if pos == KT:
                    cur_last = [i1, i2]
            return cur_last

        def stream_wlin(deps):
            pos = 0
            for nk in PL_SLABS:
                ksl = slice(pos * P, (pos + nk) * P)
                i1 = nc.sync.dma_start(
                    wlin_sb[:, pos : pos + nk, :],
                    wlin.ap()[ksl, :].rearrange("(kk p) s -> p kk s", p=P),
                )
                if pos == 0:
                    for b in deps:
                        add_dep_helper(i1.ins, b.ins, reason="class order")
                pos += nk

        jbase = 0
        for g, mts in enumerate(GROUPS):
            for k in range(KT):
                for m in mts:
                    nc.tensor.matmul(
                        ps1[m][:],
                        lhsT=wih_sb[:, k, ts(m, P)],
                        rhs=xT_sb[:, k, :],
                        start=(k == 0),
                        stop=False,
                    )
                    nc.tensor.matmul(
                        ps1[m][:],
                        lhsT=whh_sb[:, k, ts(m, P)],
                        rhs=hT_sb[:, k, :],
                        start=False,
                        stop=(k == KT - 1),
                    )

            # hnmm (feeds the doorbell store) only; the fp32 hn_out copy is
            # deferred until after the last doorbell - it is off the
            # critical path and would delay the store otherwise
            for m in mts:
                nc.scalar.activation(
                    hnmm_sb[:, m, :],
                    ps1[m][:],
                    mybir.ActivationFunctionType.Tanh,
                    bias=b1_sb[:, m : m + 1],
                )

            # group shard -> DRAM -> AllGather. Store split across gpsimd
            # and scalar so the two halves' completions retire in parallel
            # (the doorbell waits on both via the collective's input deps).
            cc1_pview = cc1_in[g].rearrange("(m p) b -> p m b", p=P)
            nm = len(mts)
            mh = nm // 2 if nm > 1 else 1
            st = nc.gpsimd.dma_start(
                cc1_pview[:, :mh, :], hnmm_sb[:, mts[0] : mts[0] + mh, :]
            )
            if nm > mh:
                nc.scalar.dma_start(
                    cc1_pview[:, mh:, :], hnmm_sb[:, mts[0] + mh : mts[-1] + 1, :]
                )
            nc.gpsimd.collective_compute(
                "AllGather",
                mybir.AluOpType.bypass,
                replica_groups=rg,
                ins=[cc1_in[g][:]],
                outs=[cc1_out[g][:]],
            )
            # next streaming class (the store->doorbell latency is a fixed
            # ~8us completion cost, so don't serialize classes behind it)
            if g + 1 < G:
                prev_last = stream_wclass(g + 1, prev_last)
            else:
                stream_wlin(prev_last)

        # own hn shard: fp32 tanh + store, off the critical path (overlaps
        # the AllGather window / phase 2)
        if mode != "fp32":
            for m in range(MS):
                nc.scalar.activation(
                    hn32_sb[:, m, :],
                    ps1[m][:],
                    mybir.ActivationFunctionType.Tanh,
                    bias=b1_sb[:, m : m + 1],
                )
        nc.scalar.dma_start(hn_out.ap().rearrange("(m p) b -> p m b", p=P), hn32_sb[:])

        # gathered j-slots -> SBUF, emitted after both doorbells so the g1
        # store/doorbell is never queue-blocked behind a copy head-wait.
        # Each group's copy is chunked across scalar and gpsimd so the first
        # phase-2 matmuls can start ~1us after the gather lands instead of
        # waiting for the full 1MB copy.
        jbase = 0
        for g, mts in enumerate(GROUPS):
            nj = NCORES * len(mts)  # j-slots this group
            cc1_view = cc1_out[g].rearrange("(rk p) b -> p rk b", p=P)
            # first chunks tiny so phase-2 matmuls resume ~0.5us after the
            # gather lands; j-slots are consumed in order by phase 2
            sizes = [1, 1, 3, 3, 4, 4] if nj == 16 else [nj // 2, nj - nj // 2]
            lo = 0
            for ci, cw in enumerate(sizes):
                eng = nc.gpsimd if ci % 2 == 0 else nc.scalar
                eng.dma_start(
                    hnT_sb[:, jbase + lo : jbase + lo + cw, :],
                    cc1_view[:, lo : lo + cw, :],
                )
                lo += cw
            jbase += nj

        # ---- phase 2: logits_c = hnT.T @ wlin (+ blin via ones-row) ----
        # hnT_sb is in j-order (group-major), and the host pre-permutes wlin's
        # rows into the same j-order, so iterating j consumes group 0's tiles
        # first (phase 2 starts as soon as AllGather 0 lands). wlin is fully
        # SBUF-resident: its 4 big DMAs queue on sync behind the phase-1
        # slabs and stream during the AllGather window when HBM is otherwise
        # idle. The bias (ones-row) matmul OPENS each accumulation group so
        # the group closes on the last j-matmul and exp can start immediately.
        ps2 = [
            ps2_pool.tile([P, SH], F32, tag=f"ps2_{mb}", name=f"ps2_{mb}")
            for mb in range(BT)
        ]
        for mb in range(BT):
            nc.tensor.matmul(
                ps2[mb][:],
                lhsT=ones_sb[:, ts(mb, P)],
                rhs=blin_sb[:],
                start=True,
                stop=False,
            )
        for j in range(KT):
            for mb in range(BT):
                nc.tensor.matmul(
                    ps2[mb][:],
                    lhsT=hnT_sb[:, j, ts(mb, P)],
                    rhs=wlin_sb[:, j, :],
                    start=False,
                    stop=(j == KT - 1),
                )

        # ---- softmax over full O (partial sums summed via AllReduce) ----
        odt = F32 if mode == "fp32" else FP16  # probs output dtype
        probs_sb = acts_pool.tile([P, BT, SH], F32)
        probs_o_sb = acts_pool.tile([P, BT, SH], odt)
        part_sb = acts_pool.tile([P, BT], F32)
        den_sb = acts_pool.tile([P, BT], F32)
        rden_sb = acts_pool.tile([P, BT], F32)

        for mb in range(BT):
            nc.scalar.activation(
                probs_sb[:, mb, :], ps2[mb][:], mybir.ActivationFunctionType.Exp
            )
            nc.vector.reduce_sum(
                part_sb[:, mb : mb + 1], probs_sb[:, mb, :], axis=mybir.AxisListType.X
            )
        nc.gpsimd.dma_start(cc2_in[:], part_sb[:])
        nc.gpsimd.collective_compute(
            "AllReduce",
            mybir.AluOpType.add,
            replica_groups=rg,
            ins=[cc2_in[:]],
            outs=[cc2_out[:]],
        )
        nc.gpsimd.dma_start(den_sb[:], cc2_out[:])
        nc.vector.reciprocal(rden_sb[:], den_sb[:])
        probs_view = probs_out.ap().rearrange("(m p) o -> p m o", p=P)
        for mb in range(BT):
            nc.vector.tensor_scalar_mul(
                probs_o_sb[:, mb, :], probs_sb[:, mb, :], rden_sb[:, mb : mb + 1]
            )
            nc.sync.dma_start(probs_view[:, mb, :], probs_o_sb[:, mb, :])


def _build(mode):
    if mode in _cache:
        return _cache[mode]
    nc = bacc.Bacc(
        "TRN2",
        target_bir_lowering=False,
        debug=False,
        num_devices=NCORES,
    )
    with tile.TileContext(nc) as tc:
        _emit(nc, tc, mode)
    nc.compile()
    _cache[mode] = nc
    return nc


def _np_dt(mode):
    if mode == "bf16":
        import ml_dtypes

        return ml_dtypes.bfloat16
    if mode == "fp16":
        return np.float16
    return np.float32


def _prep_in_maps(x, h0, w_ih, b_ih, w_hh, b_hh, w_lin, b_lin, mode):
    dt = _np_dt(mode)
    x = np.asarray(x, np.float32)
    h = np.asarray(h0, np.float32).reshape(B, H)
    w_ih = np.asarray(w_ih, np.float32)
    w_hh = np.asarray(w_hh, np.float32)
    w_lin = np.asarray(w_lin, np.float32)
    b1_full = np.asarray(b_ih, np.float32) + np.asarray(b_hh, np.float32)
    b_lin = np.asarray(b_lin, np.float32)

    xT = np.ascontiguousarray(x.T).astype(dt, copy=False)
    hT = np.ascontiguousarray(h.T).astype(dt, copy=False)

    in_maps = []
    for c in range(NCORES):
        hs = slice(c * SH, (c + 1) * SH)
        # wlin rows permuted to match hnT_sb's j-order: group-major, then
        # rank, then the group's m-tiles; global k-tile k = MS*r + m.
        wlt = np.ascontiguousarray(w_lin[hs].T).astype(dt, copy=False)
        blocks = []
        for mts in GROUPS:
            for r in range(NCORES):
                for m in mts:
                    k = MS * r + m
                    blocks.append(wlt[k * P : (k + 1) * P])
        wlt_j = np.ascontiguousarray(np.concatenate(blocks, axis=0))
        in_maps.append(
            {
                "xT": xT,
                "hT": hT,
                "wih": np.ascontiguousarray(w_ih[hs].T).astype(dt, copy=False),
                "whh": np.ascontiguousarray(w_hh[hs].T).astype(dt, copy=False),
                "wlin": wlt_j,
                "blin": np.ascontiguousarray(b_lin[hs][None, :]).astype(dt, copy=False),
                "b1": np.ascontiguousarray(b1_full[hs].reshape(MS, P).T),
            }
        )
    return in_maps


def _gather(results):
    probs = np.concatenate(
        [np.asarray(results[c]["probs_s"], np.float32) for c in range(NCORES)], axis=1
    )
    hnT = np.concatenate([results[c]["hn_s"] for c in range(NCORES)], axis=0)
    hn = np.ascontiguousarray(hnT.T)
    return probs[None, :, :], hn[None, :, :]


def run(inputs, mode=None, **spmd_kwargs):
    mode = mode or MODE
    nc = _build(mode)
    in_maps = _prep_in_maps(**inputs, mode=mode)
    res = run_bass_kernel_spmd(nc, in_maps, core_ids=list(range(NCORES)), **spmd_kwargs)
    return _gather(res.results), res


def kernel(x, h0, w_ih, b_ih, w_hh, b_hh, w_lin, b_lin):
    out, _ = run(
        dict(
            x=x, h0=h0, w_ih=w_ih, b_ih=b_ih, w_hh=w_hh, b_hh=b_hh,
            w_lin=w_lin, b_lin=b_lin,
        )
    )
    return out



# revision 32
# speedup vs baseline: 1.0683x; 1.0288x over previous
"""Trainium2 Bass kernel for a single-step Elman RNN cell + linear + softmax.

Reference computation (B=256, I=H=O=4096, fp32):
    hn     = tanh(x @ w_ih.T + b_ih + h0[0] @ w_hh.T + b_hh)      # [B, H]
    logits = hn @ w_lin.T + b_lin                                  # [B, O]
    probs  = softmax(logits, axis=-1)
    return probs[None], hn[None]

Sharding (8 cores, tensor-parallel): core c owns rows hs = [512c, 512c+512)
of H (and the same slice of O).
  Phase 1: each core computes hnT_c = tanh(W_ih[hs] @ x.T + W_hh[hs] @ h.T + b)
           as [512, 256] (H on partitions, batch on free dim), in G column
           groups; each group's [256, 256] result is AllGathered while the
           next group (and then phase 2) computes, hiding collective latency.
  Phase 2: each core computes its O-slice of logits: [256, 512] =
           (hnT k-tiles).T @ w_lin[os].T, batch on partitions.
  Softmax: exp on-chip; per-core partial row sums are AllGathered (tiny) and
           summed so every core normalizes its O-slice with the global denom.

All matmul operands are pre-transposed on the host so the contraction dim (I
resp. H) lands on SBUF partitions and every DMA is contiguous.

hnT SBUF layout: the gathered hnT k-tiles are stored in (group, rank, kk)
order, index j = g*(KT/G) + r*(KT/(G*NCORES... )); phase 2 maps global k-tile
k = 4r + 2g + kk  ->  j = g*16 + r*2 + kk (for G=2) so each group's AllGather
lands contiguously while matmuls read the right tile.
"""

import os

import numpy as np

import concourse.bass as bass
import concourse.mybir as mybir
import concourse.tile as tile
from concourse import bacc
from concourse.bass import ts
from concourse.bass_utils import run_bass_kernel_spmd
from concourse.tile_rust import add_dep_helper

NCORES = 8
B = 256
I = H = O = 4096
SH = H // NCORES  # 512: per-core shard of H / O
P = 128
KT = I // P  # 32 k-tiles
MS = SH // P  # 4 m-tiles (H-shard)
BT = B // P  # 2 batch tiles
# Phase-1 gather groups (m-tiles per AllGather). Two groups measured best:
# collective ops are latency-bound (~14us + ~6us/MB) and serialize on the
# cc stream (a doorbell can't ring until the previous op completes), so
# more groups cost more than the overlap they buy; one group delays the
# first doorbell until all of phase 1 is done.
GROUPS = [[0, 1, 2], [3]]
G = len(GROUPS)

F32 = mybir.dt.float32
BF16 = mybir.dt.bfloat16
FP16 = mybir.dt.float16

# Matmul precision mode: "fp32" (exact, 4 cyc/row), "fp16" (1 cyc/row,
# ~5e-4 rel err), "bf16" (1 cyc/row, ~3e-3 rel err).
MODE = os.environ.get("RNN_MODE", "fp16")

# k-tile groups per slab DMA: uniform medium slabs keep several DMAs in
# flight from the first issue to the end of each class (a lone in-flight
# DMA only sustains ~50 GB/s, and tiny leading slabs waste ramp time).
P1_SLABS = [4, 4, 4, 4, 4, 4, 2, 2, 2, 2]
PW_SLABS = [8, 8, 8, 8]  # later weight classes: 4 slabs per stream
PL_SLABS = [4, 4, 4, 4, 4, 4, 4, 4]  # wlin

_cache: dict = {}


def _mm_dt(mode):
    return {"fp32": F32, "bf16": BF16, "fp16": FP16}[mode]


def _emit(nc, tc, mode):
    mdt = _mm_dt(mode)

    # ---- DRAM I/O ----
    xT = nc.dram_tensor("xT", [I, B], mdt, kind="ExternalInput")
    hT = nc.dram_tensor("hT", [H, B], mdt, kind="ExternalInput")
    wih = nc.dram_tensor("wih", [I, SH], mdt, kind="ExternalInput")
    whh = nc.dram_tensor("whh", [H, SH], mdt, kind="ExternalInput")
    wlin = nc.dram_tensor("wlin", [H, SH], mdt, kind="ExternalInput")
    blin = nc.dram_tensor("blin", [1, SH], mdt, kind="ExternalInput")
    b1 = nc.dram_tensor("b1", [P, MS], F32, kind="ExternalInput")  # (b_ih+b_hh)[hs]

    odt = F32 if mode == "fp32" else FP16
    probs_out = nc.dram_tensor("probs_s", [B, SH], odt, kind="ExternalOutput")
    hn_out = nc.dram_tensor("hn_s", [SH, B], F32, kind="ExternalOutput")

    rg = [list(range(NCORES))]

    with (
        tc.tile_pool(name="const", bufs=1) as const_pool,
        tc.tile_pool(name="acts", bufs=1) as acts_pool,
        tc.tile_pool(name="ps1", bufs=1, space="PSUM") as ps1_pool,
        tc.tile_pool(name="ps2", bufs=1, space="PSUM") as ps2_pool,
        tc.tile_pool(name="dram", bufs=1, space="DRAM") as dram_pool,
    ):
        # ---- constants ----
        b1_sb = const_pool.tile([P, MS], F32)
        nc.sync.dma_start(b1_sb[:], b1.ap())
        blin_sb = const_pool.tile([1, SH], mdt)
        nc.sync.dma_start(blin_sb[:], blin.ap())
        ones_sb = const_pool.tile([1, B], mdt)
        nc.vector.memset(ones_sb[:], 1.0)

        # ---- resident activations ----
        xT_sb = acts_pool.tile([P, KT, B], mdt)
        hT_sb = acts_pool.tile([P, KT, B], mdt)
        wih_sb = acts_pool.tile([P, KT, SH], mdt)  # resident phase-1 weights
        whh_sb = acts_pool.tile([P, KT, SH], mdt)
        hnT_sb = acts_pool.tile([P, KT, B], mdt)  # gathered full hnT (j-order)
        hn32_sb = acts_pool.tile([P, MS, B], F32)  # own shard, fp32 (output)
        if mode == "fp32":
            hnmm_sb = hn32_sb
        else:
            hnmm_sb = acts_pool.tile([P, MS, B], mdt)

        # collective bounce buffers (one pair per phase-1 group)
        cc1_in = []
        cc1_out = []
        for g, mts in enumerate(GROUPS):
            gw = len(mts) * P
            t_in = dram_pool.tile([gw, B], mdt, name=f"cc1_in_{g}")
            t_out = dram_pool.tile(
                [gw * NCORES, B], mdt, addr_space="Shared", name=f"cc1_out_{g}"
            )
            cc1_in.append(t_in)
            cc1_out.append(t_out)
        # softmax denominator exchange: partition-major [P, BT] layout so the
        # 1KB store/load is 128 x 8B descriptors, AllReduce(add) so no
        # on-chip 8-way reduce is needed afterwards.
        cc2_in = dram_pool.tile([P, BT], F32)
        cc2_out = dram_pool.tile([P, BT], F32)

        # ---- phase 1, grouped; group g covers shard columns [g*GW,(g+1)*GW) ----
        ps1 = [
            ps1_pool.tile([P, B], F32, tag=f"ps1_{m}", name=f"ps1_{m}")
            for m in range(MS)
        ]

        # Phase-1 operand streaming. HBM aggregate is ~225 GB/s shared
        # pro-rata by whatever DMAs are in flight, so class sequencing is
        # everything: class g streams the wih/whh columns of gather-group g
        # (class 0 also carries x and h); the last class streams wlin. Each
        # class's first slabs carry dependency edges on the previous class's
        # last slabs so later classes never steal earlier-class bandwidth.
        # x+wih go on sync, h+whh on scalar for issue-rate headroom; wlin on
        # sync (scalar must stay clear for tanh by the time class 0 lands).
        col0 = [sum(len(m) for m in GROUPS[:g]) * P for g in range(G + 1)]
        prev_last: list = []
        pos = 0
        for nk in P1_SLABS:
            ksl = slice(pos * P, (pos + nk) * P)
            gsl = slice(col0[0], col0[1])
            i1 = nc.sync.dma_start(
                xT_sb[:, pos : pos + nk, :],
                xT.ap()[ksl, :].rearrange("(kk p) b -> p kk b", p=P),
            )
            i2 = nc.scalar.dma_start(
                hT_sb[:, pos : pos + nk, :],
                hT.ap()[ksl, :].rearrange("(kk p) b -> p kk b", p=P),
            )
            i3 = nc.sync.dma_start(
                wih_sb[:, pos : pos + nk, gsl],
                wih.ap()[ksl, gsl].rearrange("(kk p) s -> p kk s", p=P),
            )
            i4 = nc.scalar.dma_start(
                whh_sb[:, pos : pos + nk, gsl],
                whh.ap()[ksl, gsl].rearrange("(kk p) s -> p kk s", p=P),
            )
            pos += nk
            if pos == KT:
                prev_last = [i1, i2, i3, i4]
        wlin_sb = acts_pool.tile([P, KT, SH], mdt)

        def stream_wclass(g, deps):
            # stream wih/whh columns of group g in slabs; first slabs
            # depend on `deps` (previous class's last slabs + the doorbell
            # store, so the tiny store DMA isn't starved of bandwidth)
            gsl = slice(col0[g], col0[g + 1])
            cur_last = []
            pos = 0
            for nk in PW_SLABS:
                ksl = slice(pos * P, (pos + nk) * P)
                i1 = nc.sync.dma_start(
                    wih_sb[:, pos : pos + nk, gsl],
                    wih.ap()[ksl, gsl].rearrange("(kk p) s -> p kk s", p=P),
                )
                i2 = nc.sync.dma_start(
                    whh_sb[:, pos : pos + nk, gsl],
                    whh.ap()[ksl, gsl].rearrange("(kk p) s -> p kk s", p=P),
                )
                if pos == 0:
                    for a in deps:
                        add_dep_helper(i1.ins, a.ins, reason="class order")
                        add_dep_helper(i2.ins, a.ins, reason="class order")
                pos += nk
                if pos == KT:
                    cur_last = [i1, i2]
            return cur_last

        def stream_wlin(deps):
            pos = 0
            for nk in PL_SLABS:
                ksl = slice(pos * P, (pos + nk) * P)
                i1 = nc.sync.dma_start(
                    wlin_sb[:, pos : pos + nk, :],
                    wlin.ap()[ksl, :].rearrange("(kk p) s -> p kk s", p=P),
                )
                if pos == 0:
                    for b in deps:
                        add_dep_helper(i1.ins, b.ins, reason="class order")
                pos += nk

        jbase = 0
        for g, mts in enumerate(GROUPS):
            for k in range(KT):
                for m in mts:
                    nc.tensor.matmul(
                        ps1[m][:],
                        lhsT=wih_sb[:, k, ts(m, P)],
                        rhs=xT_sb[:, k, :],
                        start=(k == 0),
                        stop=False,
                    )
                    nc.tensor.matmul(
                        ps1[m][:],
                        lhsT=whh_sb[:, k, ts(m, P)],
                        rhs=hT_sb[:, k, :],
                        start=False,
                        stop=(k == KT - 1),
                    )

            # hnmm (feeds the doorbell store) only; the fp32 hn_out copy is
            # deferred until after the last doorbell - it is off the
            # critical path and would delay the store otherwise
            for m in mts:
                nc.scalar.activation(
                    hnmm_sb[:, m, :],
                    ps1[m][:],
                    mybir.ActivationFunctionType.Tanh,
                    bias=b1_sb[:, m : m + 1],
                )

            # group shard -> DRAM -> AllGather. Store split across gpsimd
            # and scalar so the two halves' completions retire in parallel
            # (the doorbell waits on both via the collective's input deps).
            cc1_pview = cc1_in[g].rearrange("(m p) b -> p m b", p=P)
            nm = len(mts)
            mh = nm // 2 if nm > 1 else 1
            st = nc.gpsimd.dma_start(
                cc1_pview[:, :mh, :], hnmm_sb[:, mts[0] : mts[0] + mh, :]
            )
            if nm > mh:
                nc.scalar.dma_start(
                    cc1_pview[:, mh:, :], hnmm_sb[:, mts[0] + mh : mts[-1] + 1, :]
                )
            nc.gpsimd.collective_compute(
                "AllGather",
                mybir.AluOpType.bypass,
                replica_groups=rg,
                ins=[cc1_in[g][:]],
                outs=[cc1_out[g][:]],
            )
            # next streaming class (the store->doorbell latency is a fixed
            # ~8us completion cost, so don't serialize classes behind it)
            if g + 1 < G:
                prev_last = stream_wclass(g + 1, prev_last)
            else:
                stream_wlin(prev_last)

        # own hn shard: fp32 tanh + store, off the critical path (overlaps
        # the AllGather window / phase 2)
        if mode != "fp32":
            for m in range(MS):
                nc.scalar.activation(
                    hn32_sb[:, m, :],
                    ps1[m][:],
                    mybir.ActivationFunctionType.Tanh,
                    bias=b1_sb[:, m : m + 1],
                )
        nc.scalar.dma_start(hn_out.ap().rearrange("(m p) b -> p m b", p=P), hn32_sb[:])

        # gathered j-slots -> SBUF, emitted after both doorbells so the g1
        # store/doorbell is never queue-blocked behind a copy head-wait.
        # Each group's copy is chunked across scalar and gpsimd so the first
        # phase-2 matmuls can start ~1us after the gather lands instead of
        # waiting for the full 1MB copy.
        jbase = 0
        for g, mts in enumerate(GROUPS):
            nj = NCORES * len(mts)  # j-slots this group
            cc1_view = cc1_out[g].rearrange("(rk p) b -> p rk b", p=P)
            # first chunks tiny so phase-2 matmuls resume ~0.5us after the
            # gather lands; j-slots are consumed in order by phase 2
            sizes = [1, 1, 3, 3]
            while sum(sizes) < nj:
                sizes.append(min(4, nj - sum(sizes)))
            lo = 0
            for ci, cw in enumerate(sizes):
                eng = nc.gpsimd if ci % 2 == 0 else nc.scalar
                eng.dma_start(
                    hnT_sb[:, jbase + lo : jbase + lo + cw, :],
                    cc1_view[:, lo : lo + cw, :],
                )
                lo += cw
            jbase += nj

        # ---- phase 2: logits_c = hnT.T @ wlin (+ blin via ones-row) ----
        # hnT_sb is in j-order (group-major), and the host pre-permutes wlin's
        # rows into the same j-order, so iterating j consumes group 0's tiles
        # first (phase 2 starts as soon as AllGather 0 lands). wlin is fully
        # SBUF-resident: its 4 big DMAs queue on sync behind the phase-1
        # slabs and stream during the AllGather window when HBM is otherwise
        # idle. The bias (ones-row) matmul OPENS each accumulation group so
        # the group closes on the last j-matmul and exp can start immediately.
        ps2 = [
            ps2_pool.tile([P, SH], F32, tag=f"ps2_{mb}", name=f"ps2_{mb}")
            for mb in range(BT)
        ]
        for mb in range(BT):
            nc.tensor.matmul(
                ps2[mb][:],
                lhsT=ones_sb[:, ts(mb, P)],
                rhs=blin_sb[:],
                start=True,
                stop=False,
            )
        for j in range(KT):
            for mb in range(BT):
                nc.tensor.matmul(
                    ps2[mb][:],
                    lhsT=hnT_sb[:, j, ts(mb, P)],
                    rhs=wlin_sb[:, j, :],
                    start=False,
                    stop=(j == KT - 1),
                )

        # ---- softmax over full O (partial sums summed via AllReduce) ----
        odt = F32 if mode == "fp32" else FP16  # probs output dtype
        probs_sb = acts_pool.tile([P, BT, SH], F32)
        probs_o_sb = acts_pool.tile([P, BT, SH], odt)
        part_sb = acts_pool.tile([P, BT], F32)
        den_sb = acts_pool.tile([P, BT], F32)
        rden_sb = acts_pool.tile([P, BT], F32)

        for mb in range(BT):
            nc.scalar.activation(
                probs_sb[:, mb, :], ps2[mb][:], mybir.ActivationFunctionType.Exp
            )
            nc.vector.reduce_sum(
                part_sb[:, mb : mb + 1], probs_sb[:, mb, :], axis=mybir.AxisListType.X
            )
        nc.gpsimd.dma_start(cc2_in[:], part_sb[:])
        nc.gpsimd.collective_compute(
            "AllReduce",
            mybir.AluOpType.add,
            replica_groups=rg,
            ins=[cc2_in[:]],
            outs=[cc2_out[:]],
        )
        nc.gpsimd.dma_start(den_sb[:], cc2_out[:])
        nc.vector.reciprocal(rden_sb[:], den_sb[:])
        probs_view = probs_out.ap().rearrange("(m p) o -> p m o", p=P)
        for mb in range(BT):
            nc.vector.tensor_scalar_mul(
                probs_o_sb[:, mb, :], probs_sb[:, mb, :], rden_sb[:, mb : mb + 1]
            )
            nc.sync.dma_start(probs_view[:, mb, :], probs_o_sb[:, mb, :])


def _build(mode):
    if mode in _cache:
        return _cache[mode]
    nc = bacc.Bacc(
        "TRN2",
        target_bir_lowering=False,
        debug=False,
        num_devices=NCORES,
    )
    with tile.TileContext(nc) as tc:
        _emit(nc, tc, mode)
    nc.compile()
    _cache[mode] = nc
    return nc


def _np_dt(mode):
    if mode == "bf16":
        import ml_dtypes

        return ml_dtypes.bfloat16
    if mode == "fp16":
        return np.float16
    return np.float32


def _prep_in_maps(x, h0, w_ih, b_ih, w_hh, b_hh, w_lin, b_lin, mode):
    dt = _np_dt(mode)
    x = np.asarray(x, np.float32)
    h = np.asarray(h0, np.float32).reshape(B, H)
    w_ih = np.asarray(w_ih, np.float32)
    w_hh = np.asarray(w_hh, np.float32)
    w_lin = np.asarray(w_lin, np.float32)
    b1_full = np.asarray(b_ih, np.float32) + np.asarray(b_hh, np.float32)
    b_lin = np.asarray(b_lin, np.float32)

    xT = np.ascontiguousarray(x.T).astype(dt, copy=False)
    hT = np.ascontiguousarray(h.T).astype(dt, copy=False)

    in_maps = []
    for c in range(NCORES):
        hs = slice(c * SH, (c + 1) * SH)
        # wlin rows permuted to match hnT_sb's j-order: group-major, then
        # rank, then the group's m-tiles; global k-tile k = MS*r + m.
        wlt = np.ascontiguousarray(w_lin[hs].T).astype(dt, copy=False)
        blocks = []
        for mts in GROUPS:
            for r in range(NCORES):
                for m in mts:
                    k = MS * r + m
                    blocks.append(wlt[k * P : (k + 1) * P])
        wlt_j = np.ascontiguousarray(np.concatenate(blocks, axis=0))
        in_maps.append(
            {
                "xT": xT,
                "hT": hT,
                "wih": np.ascontiguousarray(w_ih[hs].T).astype(dt, copy=False),
                "whh": np.ascontiguousarray(w_hh[hs].T).astype(dt, copy=False),
                "wlin": wlt_j,
                "blin": np.ascontiguousarray(b_lin[hs][None, :]).astype(dt, copy=False),
                "b1": np.ascontiguousarray(b1_full[hs].reshape(MS, P).T),
            }
        )
    return in_maps


def _gather(results):
    probs = np.concatenate(
        [np.asarray(results[c]["probs_s"], np.float32) for c in range(NCORES)], axis=1
    )
    hnT = np.concatenate([results[c]["hn_s"] for c in range(NCORES)], axis=0)
    hn = np.ascontiguousarray(hnT.T)
    return probs[None, :, :], hn[None, :, :]


def run(inputs, mode=None, **spmd_kwargs):
    mode = mode or MODE
    nc = _build(mode)
    in_maps = _prep_in_maps(**inputs, mode=mode)
    res = run_bass_kernel_spmd(nc, in_maps, core_ids=list(range(NCORES)), **spmd_kwargs)
    return _gather(res.results), res


def kernel(x, h0, w_ih, b_ih, w_hh, b_hh, w_lin, b_lin):
    out, _ = run(
        dict(
            x=x, h0=h0, w_ih=w_ih, b_ih=b_ih, w_hh=w_hh, b_hh=b_hh,
            w_lin=w_lin, b_lin=b_lin,
        )
    )
    return out

